# revision 8
# baseline (speedup 1.0000x reference)
"""Trainium2 Bass kernel for CrossAttention (B=2,S=2048,D=1024,H=16,HD=64,PAST=2048).

Sharding: 8 cores = 2 batch groups x 4 row-shards (512 query rows each).
Per core: q/k/v projections for its rows (fp32r matmuls off PE-transposed
xT/encT), two packed AllGathers distribute transposed-K and [V|1]-staged
tiles (past is head-sharded for staging), then per-head attention in the
scores-transposed layout: softmax denominator rides row 64 of the AV psum
via a fused ones column; normalization folds into the ctx copy; output
projection Wo with K=1 ones bias-fold. Host concatenates past_k/past_v
with the gathered new K/V (pure unshard/concat).
"""
import functools

import numpy as np

B, S, D, H = 2, 2048, 1024, 16
HD = D // H          # 64
PAST = 2048
L = PAST + S         # 4096
NC = 8               # cores
RPC = S * B // NC    # 512 rows per core
GSZ = 4              # cores per batch group
HL = H // GSZ        # 4 heads staged per core

KT_PAST_ELEMS = HD * PAST            # 131072 per head
V_PAST_ELEMS = PAST * (HD + 1)       # 133120 per head
PAST_BLOB = KT_PAST_ELEMS + V_PAST_ELEMS   # 264192
KT_NEW_ELEMS = D * RPC               # 524288
V_NEW_ELEMS = RPC * H * (HD + 1)     # 532480
NEW_BLOB = KT_NEW_ELEMS + V_NEW_ELEMS      # 1056768

NKT = L // 128       # 32 key tiles per head
PKT = PAST // 128    # 16 past key tiles


@functools.lru_cache(maxsize=1)
def _build():
    from concourse import bacc
    import concourse.mybir as mybir
    import concourse.tile as tile
    from concourse.masks import make_identity

    F32 = mybir.dt.float32
    F32R = mybir.dt.float32r
    AF = mybir.ActivationFunctionType

    nc = bacc.Bacc("TRN2", target_bir_lowering=False, debug=False)

    # ---------------- I/O ----------------
    x_in = nc.dram_tensor("x", [RPC, D], F32, kind="ExternalInput")
    enc_in = nc.dram_tensor("enc", [RPC, D], F32, kind="ExternalInput")
    wq_in = nc.dram_tensor("wq", [D, D], F32R, kind="ExternalInput")
    wk_in = nc.dram_tensor("wk", [D, D], F32R, kind="ExternalInput")
    wv_in = nc.dram_tensor("wv", [D, D], F32R, kind="ExternalInput")
    wo_in = nc.dram_tensor("wo", [D, D], F32R, kind="ExternalInput")
    bq_in = nc.dram_tensor("bq", [D], F32, kind="ExternalInput")
    bk_in = nc.dram_tensor("bk", [D], F32, kind="ExternalInput")
    bkr_in = nc.dram_tensor("bkr", [1, D], F32R, kind="ExternalInput")
    bvr_in = nc.dram_tensor("bvr", [1, D], F32R, kind="ExternalInput")
    bor_in = nc.dram_tensor("bor", [1, D], F32R, kind="ExternalInput")
    pk_in = nc.dram_tensor("pk", [HL, PAST, HD], F32, kind="ExternalInput")
    pv_in = nc.dram_tensor("pv", [HL, PAST, HD], F32, kind="ExternalInput")

    k_out = nc.dram_tensor("k_new", [RPC, D], F32, kind="ExternalOutput")
    v_out = nc.dram_tensor("v_new", [RPC, D], F32, kind="ExternalOutput")
    o_out = nc.dram_tensor("attn_out", [RPC, D], F32, kind="ExternalOutput")

    # ------------- internal DRAM -------------
    stage_past = nc.dram_tensor("stage_past", [HL, PAST_BLOB], F32R)
    past_all = nc.dram_tensor("past_all", [H, PAST_BLOB], F32R)
    stage_new = nc.dram_tensor("stage_new", [NEW_BLOB], F32R)
    rcp_dram = nc.dram_tensor("rcp_dram", [H, RPC], F32)
    new_all = nc.dram_tensor("new_all", [GSZ, NEW_BLOB], F32R)

    groups = [[0, 1, 2, 3], [4, 5, 6, 7]]

    with tile.TileContext(nc) as tc:
      with tc.tile_pool(name="persist", bufs=1) as pers:
        with (
            tc.tile_pool(name="ps", bufs=3, space="PSUM") as ps,
            tc.tile_pool(name="p1", bufs=2) as p1,
            tc.tile_pool(name="wpool", bufs=9) as wpool,
            tc.tile_pool(name="projT", bufs=3) as projTp,
            tc.tile_pool(name="knat", bufs=3) as knatp,
        ):
            ident = pers.tile([128, 128], F32)
            make_identity(nc, ident)
            ones_row = pers.tile([1, 128], F32R)
            nc.vector.memset(ones_row[:].bitcast(F32), 1.0)
            bq_part = pers.tile([128, 8], F32)
            nc.sync.dma_start(bq_part[:], bq_in.rearrange("(o p) -> p o", p=128))
            bk_part = pers.tile([128, 8], F32)
            nc.sync.dma_start(bk_part[:], bk_in.rearrange("(o p) -> p o", p=128))
            bkr = pers.tile([1, D], F32R)
            nc.sync.dma_start(bkr[:], bkr_in[:])
            bvr = pers.tile([1, D], F32R)
            nc.sync.dma_start(bvr[:], bvr_in[:])
            bor = pers.tile([1, D], F32R)
            nc.sync.dma_start(bor[:], bor_in[:])

            qT_all = pers.tile([128, 8, RPC], F32R)
            ctxT_all = pers.tile([128, 8, RPC], F32R)
            xT = pers.tile([128, 8, RPC], F32R)
            encT = pers.tile([128, 8, RPC], F32R)

            # ============ Phase 0: past staging (feeds AG1 early) ============
            for hl in range(HL):
                pk_sb = p1.tile([128, PKT, HD], F32, tag="pk")
                nc.sync.dma_start(
                    pk_sb[:], pk_in[hl].rearrange("(t p) c -> p t c", p=128))
                kTp = p1.tile([64, PKT, 128], F32R, tag="kTp")
                for t in range(PKT):
                    pt = ps.tile([128, 2, 512], F32, tag="s")
                    nc.tensor.transpose(pt[:HD, 0, :128], pk_sb[:, t, :], ident[:])
                    nc.vector.tensor_copy(kTp[:, t, :], pt[:HD, 0, :128])
                nc.sync.dma_start(
                    stage_past[hl, 0:KT_PAST_ELEMS].rearrange(
                        "(p t f) -> p t f", p=64, t=PKT, f=128),
                    kTp[:])

                pv_sb = p1.tile([128, PKT, HD], F32, tag="pv")
                nc.sync.dma_start(
                    pv_sb[:], pv_in[hl].rearrange("(t p) c -> p t c", p=128))
                v65p = p1.tile([128, PKT, HD + 1], F32R, tag="v65p")
                nc.vector.tensor_copy(v65p[:, :, 0:HD], pv_sb[:])
                nc.vector.memset(v65p[:, :, HD:HD + 1].bitcast(F32), 1.0)
                nc.sync.dma_start(
                    stage_past[hl, KT_PAST_ELEMS:PAST_BLOB].rearrange(
                        "(t p c) -> p t c", t=PKT, p=128, c=HD + 1),
                    v65p[:])

            nc.gpsimd.collective_compute(
                "AllGather", mybir.AluOpType.bypass, replica_groups=groups,
                ins=[stage_past[:]], outs=[past_all[:]])

            # ============ Phase 1: xT / encT via PE transpose ============
            for src_dram, dstT in ((x_in, xT), (enc_in, encT)):
                for rt in range(RPC // 128):
                    xs = p1.tile([128, D], F32, tag="xe")
                    nc.sync.dma_start(xs[:], src_dram[rt * 128:(rt + 1) * 128, :])
                    for t2 in range(8):
                        pt = ps.tile([128, 2, 512], F32, tag="s")
                        nc.tensor.transpose(
                            pt[:, 0, :128], xs[:, t2 * 128:(t2 + 1) * 128], ident[:])
                        nc.vector.tensor_copy(
                            dstT[:, t2, rt * 128:(rt + 1) * 128], pt[:, 0, :128])

            # ============ Phase 2: projections ============
            # qT/kT: [chan, rows] = Wq[:,chan-tile].T @ xT ; bias per-partition
            wq_tiles = []
            for t in range(8):
                wt = wpool.tile([128, D], F32R, tag="w")
                nc.sync.dma_start(
                    wt[:], wq_in.rearrange("(o p) c -> p o c", p=128)[:, t, :])
                wq_tiles.append(wt)
            for t in range(8):
                pq = ps.tile([128, 2, 512], F32, tag="s")
                for kt8 in range(8):
                    nc.tensor.matmul(
                        pq[:, 0, :], wq_tiles[kt8][:, t * 128:(t + 1) * 128],
                        xT[:, kt8, :], start=(kt8 == 0), stop=(kt8 == 7))
                nc.vector.tensor_scalar_add(
                    qT_all[:, t, :], pq[:, 0, :], bq_part[:, t:t + 1])

            wk_tiles = []
            for t in range(8):
                wt = wpool.tile([128, D], F32R, tag="w")
                nc.sync.dma_start(
                    wt[:], wk_in.rearrange("(o p) c -> p o c", p=128)[:, t, :])
                wk_tiles.append(wt)
            kT_stage_view = stage_new[0:KT_NEW_ELEMS].rearrange(
                "(o p f) -> p o f", o=8, p=128, f=RPC)
            for t in range(8):
                pk_ = ps.tile([128, 2, 512], F32, tag="s")
                for kt8 in range(8):
                    nc.tensor.matmul(
                        pk_[:, 0, :], wk_tiles[kt8][:, t * 128:(t + 1) * 128],
                        encT[:, kt8, :], start=(kt8 == 0), stop=(kt8 == 7))
                kT_sb = projTp.tile([128, RPC], F32R, tag="projT")
                nc.vector.tensor_scalar_add(
                    kT_sb[:], pk_[:, 0, :], bk_part[:, t:t + 1])
                nc.sync.dma_start(kT_stage_view[:, t, :], kT_sb[:])

            # k_nat / v_nat: [rows, chan] = encT.T @ Wk + ones.T@bk_row
            v_stage_view = stage_new[KT_NEW_ELEMS:NEW_BLOB].rearrange(
                "(rt p t c) -> p rt t c", rt=4, p=128, t=H, c=HD + 1)
            for w_in_, brow, out_dram, is_v in (
                    (wk_in, bkr, k_out, False), (wv_in, bvr, v_out, True)):
                w_tiles = []
                for t in range(8):
                    wt = wpool.tile([128, D], F32R, tag="w")
                    nc.sync.dma_start(
                        wt[:], w_in_.rearrange("(o p) c -> p o c", p=128)[:, t, :])
                    w_tiles.append(wt)
                for rt in range(RPC // 128):
                    if is_v:
                        v65n = knatp.tile([128, H, HD + 1], F32R, tag="v65n")
                        nc.vector.memset(
                            v65n[:, :, HD:HD + 1].bitcast(F32), 1.0)
                    for nt in range(2):
                        pn = ps.tile([128, 2, 512], F32, tag="s")
                        for kt8 in range(8):
                            nc.tensor.matmul(
                                pn[:, 0, :],
                                encT[:, kt8, rt * 128:(rt + 1) * 128],
                                w_tiles[kt8][:, nt * 512:(nt + 1) * 512],
                                start=(kt8 == 0), stop=False)
                        nc.tensor.matmul(
                            pn[:, 0, :], ones_row[:],
                            brow[:, nt * 512:(nt + 1) * 512],
                            start=False, stop=True)
                        nat = knatp.tile([128, 512], F32, tag="knat")
                        nc.vector.tensor_copy(nat[:], pn[:, 0, :])
                        nc.sync.dma_start(
                            out_dram[rt * 128:(rt + 1) * 128,
                                     nt * 512:(nt + 1) * 512], nat[:])
                        if is_v:
                            nc.vector.tensor_copy(
                                v65n[:, nt * 8:(nt + 1) * 8, 0:HD],
                                pn[:, 0, :].rearrange(
                                    "p (h c) -> p h c", h=8, c=HD))
                    if is_v:
                        nc.sync.dma_start(
                            v_stage_view[:, rt, :, :], v65n[:])

            nc.gpsimd.collective_compute(
                "AllGather", mybir.AluOpType.bypass, replica_groups=groups,
                ins=[stage_new[:]], outs=[new_all[:]])

        # ============ Phase 3: attention (new pools reuse p1 space) ============
        with (
            tc.tile_pool(name="kv", bufs=2) as kvp,
            tc.tile_pool(name="v65", bufs=3) as v65pool,
            tc.tile_pool(name="exp", bufs=3) as expp,
            tc.tile_pool(name="rcp", bufs=4) as rcpp,
            tc.tile_pool(name="wop", bufs=8) as wop,
            tc.tile_pool(name="oout", bufs=2) as ooutp,
            tc.tile_pool(name="ps2", bufs=3, space="PSUM") as ps2,
            tc.tile_pool(name="psctx", bufs=2, space="PSUM") as psctx,
        ):
            v65_new_view = [
                new_all[r, KT_NEW_ELEMS:NEW_BLOB].rearrange(
                    "(kt kp t c) -> kp kt t c", kt=4, kp=128, t=H, c=HD + 1)
                for r in range(GSZ)]

            for hp in range(H // 2):
                kT_pair = kvp.tile([128, L], F32R, tag="kT")
                v65_sb = []
                for j in range(2):
                    h = 2 * hp + j
                    nc.sync.dma_start(
                        kT_pair[64 * j:64 * j + 64, 0:PAST],
                        past_all[h, 0:KT_PAST_ELEMS].rearrange(
                            "(p f) -> p f", p=64))
                    for r in range(GSZ):
                        nc.sync.dma_start(
                            kT_pair[64 * j:64 * j + 64,
                                    PAST + r * RPC:PAST + (r + 1) * RPC],
                            new_all[r, h * HD * RPC:(h + 1) * HD * RPC]
                            .rearrange("(p f) -> p f", p=64))
                    v65 = v65pool.tile([128, NKT, HD + 1], F32R, tag="v65")
                    nc.sync.dma_start(
                        v65[:, 0:PKT, :],
                        past_all[h, KT_PAST_ELEMS:PAST_BLOB].rearrange(
                            "(t p c) -> p t c", t=PKT, p=128, c=HD + 1))
                    for r in range(GSZ):
                        nc.sync.dma_start(
                            v65[:, PKT + 4 * r:PKT + 4 * r + 4, :],
                            v65_new_view[r][:, :, h, :])
                    v65_sb.append(v65)

                ps_ctx = [psctx.tile([HD + 1, RPC], F32, tag="ctx", name=f"ctx{hp}_{jj}")
                          for jj in range(2)]
                for ktb in range(NKT // 2):   # batches of 2 key tiles
                    for j in range(2):
                        ps_s = ps2.tile([128, 2, 512], F32, tag="s2")
                        for u in range(2):
                            kt = 2 * ktb + u
                            nc.tensor.matmul(
                                ps_s[:, u, :],
                                kT_pair[64 * j:64 * j + 64,
                                        kt * 128:(kt + 1) * 128],
                                qT_all[64 * j:64 * j + 64, hp, :],
                                start=True, stop=True)
                        ex = expp.tile([128, 2, 512], F32R, tag="exp")
                        nc.scalar.activation(ex[:], ps_s[:], AF.Exp, scale=0.125)
                        for u in range(2):
                            kt = 2 * ktb + u
                            nc.tensor.matmul(
                                ps_ctx[j][:], v65_sb[j][:, kt, :], ex[:, u, :],
                                start=(kt == 0), stop=(kt == NKT - 1))

                for j in range(2):
                    h = 2 * hp + j
                    ctx_sb = rcpp.tile([HD + 1, RPC], F32, tag="ctxsb")
                    nc.vector.tensor_copy(ctx_sb[:], ps_ctx[j][:])
                    rcp = rcpp.tile([1, RPC], F32, tag="rcp")
                    nc.vector.reciprocal(rcp[:], ctx_sb[HD:HD + 1, :])
                    nc.sync.dma_start(rcp_dram[h:h + 1, :], rcp[:])
                    rcpb = rcpp.tile([64, RPC], F32, tag="rcpb")
                    nc.sync.dma_start(
                        rcpb[:], rcp_dram[h:h + 1, :].to_broadcast((64, RPC)))
                    nc.vector.tensor_mul(
                        ctxT_all[64 * j:64 * j + 64, hp, :],
                        ctx_sb[0:HD, :], rcpb[:])

            # ============ Phase 4: output projection ============
            wo_tiles = []
            for t in range(8):
                wt = wop.tile([128, D], F32R, tag="wo")
                nc.sync.dma_start(
                    wt[:], wo_in.rearrange("(o p) c -> p o c", p=128)[:, t, :])
                wo_tiles.append(wt)
            for rt in range(RPC // 128):
                for ont in range(2):
                    po = ps2.tile([128, 2, 512], F32, tag="s2")
                    for t in range(8):
                        nc.tensor.matmul(
                            po[:, 0, :],
                            ctxT_all[:, t, rt * 128:(rt + 1) * 128],
                            wo_tiles[t][:, ont * 512:(ont + 1) * 512],
                            start=(t == 0), stop=False)
                    nc.tensor.matmul(
                        po[:, 0, :], ones_row[:],
                        bor[:, ont * 512:(ont + 1) * 512],
                        start=False, stop=True)
                    osb = ooutp.tile([128, 512], F32, tag="osb")
                    nc.vector.tensor_copy(osb[:], po[:, 0, :])
                    nc.sync.dma_start(
                        o_out[rt * 128:(rt + 1) * 128,
                              ont * 512:(ont + 1) * 512], osb[:])

    nc.compile()
    return nc


def _make_in_maps(input, encoder_out, past_k, past_v, Wq, bq, Wk, bk, Wv, bv,
                  Wo, bo):
    f = np.float32
    asf = lambda a: np.ascontiguousarray(np.asarray(a), dtype=f)
    input, encoder_out = asf(input), asf(encoder_out)
    past_k, past_v = asf(past_k), asf(past_v)
    Wq, Wk, Wv, Wo = asf(Wq), asf(Wk), asf(Wv), asf(Wo)
    bq, bk, bv, bo = asf(bq), asf(bk), asf(bv), asf(bo)
    in_maps = []
    for c in range(NC):
        b, g = c // GSZ, c % GSZ
        in_maps.append({
            "x": np.ascontiguousarray(input[b, g * RPC:(g + 1) * RPC, :]),
            "enc": np.ascontiguousarray(encoder_out[b, g * RPC:(g + 1) * RPC, :]),
            "wq": Wq, "wk": Wk, "wv": Wv, "wo": Wo,
            "bq": bq, "bk": bk,
            "bkr": bk.reshape(1, -1), "bvr": bv.reshape(1, -1),
            "bor": bo.reshape(1, -1),
            "pk": np.ascontiguousarray(past_k[b, g * HL:(g + 1) * HL]),
            "pv": np.ascontiguousarray(past_v[b, g * HL:(g + 1) * HL]),
        })
    return in_maps


def _run(in_maps):
    from concourse.bass_utils import run_bass_kernel_spmd
    nc = _build()
    return run_bass_kernel_spmd(nc, in_maps, core_ids=list(range(NC)))


def _assemble(results, past_k, past_v):
    attn_out = np.empty((B, S, D), dtype=np.float32)
    k_new = np.empty((B, H, S, HD), dtype=np.float32)
    v_new = np.empty((B, H, S, HD), dtype=np.float32)
    for c in range(NC):
        b, g = c // GSZ, c % GSZ
        r = results[c]
        attn_out[b, g * RPC:(g + 1) * RPC, :] = r["attn_out"]
        k_new[b, :, g * RPC:(g + 1) * RPC, :] = (
            r["k_new"].reshape(RPC, H, HD).transpose(1, 0, 2))
        v_new[b, :, g * RPC:(g + 1) * RPC, :] = (
            r["v_new"].reshape(RPC, H, HD).transpose(1, 0, 2))
    k = np.concatenate(
        [np.asarray(past_k, dtype=np.float32), k_new], axis=2)
    v = np.concatenate(
        [np.asarray(past_v, dtype=np.float32), v_new], axis=2)
    return attn_out, k, v


def kernel(input, encoder_out, past_k, past_v, Wq, bq, Wk, bk, Wv, bv, Wo, bo):
    in_maps = _make_in_maps(input, encoder_out, past_k, past_v, Wq, bq, Wk,
                            bk, Wv, bv, Wo, bo)
    res = _run(in_maps)
    return _assemble(res.results, past_k, past_v)
